# revision 1
# baseline (speedup 1.0000x reference)
"""Causal self-attention (single head, S=4096, D=1024) on 8 TRN2 NeuronCores.

Strategy (sequence-parallel, instruction-count-optimized):
  - Core c owns contiguous query rows [512c, 512(c+1)) and computes the
    K/V projections for the same rows; K^T/V are AllGathered in bf16.
  - Scores are computed TRANSPOSED (S^T[j, i], keys on partitions): softmax
    needs no max-subtraction (scores are O(5)), exp(S^T) feeds the PV matmul
    directly as lhsT (no transposes), row sums come from a ones-vector matmul
    accumulated in PSUM across the whole loop.
  - Every core runs the same fully-static program over all 8 key windows
    (window W = AG rank block W); causality and the padded (future) windows
    are handled by per-core multiplicative mask streams prepared on the host.
"""

import numpy as np
import ml_dtypes

S = 4096
D = 1024
N_CORES = 8
P = 128
SH = 512              # per-core query shard rows / key window
N_WIN = 8
KT_ELEMS = D * SH           # per-rank kT block elements in AG buffer
V_ELEMS = SH * D
RANK_ELEMS = KT_ELEMS + V_ELEMS
AG_OUT_ELEMS = N_CORES * RANK_ELEMS
SCALE = 1.0 / 32.0          # 1/sqrt(D)

_CACHE = {}


def _build(win_mult=1, pv_split=True, st_split=True, dma_split=2, wbufs=2, ptbufs=3, mkbufs=2, parts=frozenset({'sc','pv','act','msk','rs','dma','acc','ag'})):
    import concourse.bass as bass
    import concourse.mybir as mybir
    import concourse.tile as tile
    from concourse import bacc

    bf16 = mybir.dt.bfloat16
    f32 = mybir.dt.float32

    nc = bacc.Bacc("TRN2", target_bir_lowering=False, debug=False,
                   num_devices=N_CORES)

    # ---- per-core I/O ----
    wq = nc.dram_tensor("wq", [P, 8, D], bf16, kind="ExternalInput")
    wk = nc.dram_tensor("wk", [P, 8, D], bf16, kind="ExternalInput")
    wv = nc.dram_tensor("wv", [P, 8, D], bf16, kind="ExternalInput")
    xs = nc.dram_tensor("xs", [P, 8, SH], bf16, kind="ExternalInput")  # x^T shard
    maskd = nc.dram_tensor("mask", [N_WIN, 2, P, 2, SH], bf16, kind="ExternalInput")
    onesd = nc.dram_tensor("ones", [P, 1], bf16, kind="ExternalInput")
    outd = nc.dram_tensor("out", [SH, D], f32, kind="ExternalOutput")

    agin = nc.dram_tensor("agin", [1, RANK_ELEMS], bf16)
    agout = nc.dram_tensor("agout", [1, AG_OUT_ELEMS], bf16, addr_space="Shared")
    rs_dram = nc.dram_tensor("rs_dram", [1, SH], f32)

    with tile.TileContext(nc) as tc:
        with tc.tile_pool(name="wpool", bufs=3) as wpool, \
             tc.tile_pool(name="xpool", bufs=1) as xpool, \
             tc.tile_pool(name="qt", bufs=1) as qtpool, \
             tc.tile_pool(name="stage", bufs=3) as stage, \
             tc.tile_pool(name="consts", bufs=1) as consts, \
             tc.tile_pool(name="accs", bufs=1) as accs:

            # ---------------- Phase 1: projections ----------------
            xs_sb = xpool.tile([P, 8, SH], bf16, name="xs_sb")
            wk_sb = wpool.tile([P, 8, D], bf16, name="wk_sb", tag="w")
            wv_sb = wpool.tile([P, 8, D], bf16, name="wv_sb", tag="w")
            wq_sb = wpool.tile([P, 8, D], bf16, name="wq_sb", tag="w")
            for ko in range(8):
                nc.sync.dma_start(xs_sb[:, ko, :], xs[:, ko, :])
                nc.sync.dma_start(wk_sb[:, ko, :], wk[:, ko, :])
            for ko in range(8):
                nc.sync.dma_start(wv_sb[:, ko, :], wv[:, ko, :])
                nc.sync.dma_start(wq_sb[:, ko, :], wq[:, ko, :])

            with tc.tile_pool(name="pps", bufs=2, space="PSUM") as pps, \
                 tc.tile_pool(name="ppsv", bufs=2, space="PSUM") as ppsv:
                # kT_c: [8 dko][128 dp][512 j] into agin[0 : KT_ELEMS]
                for dt2 in range(4):          # two d-tiles per psum tile
                    ps = pps.tile([P, 2, SH], f32, name=f"kt_ps{dt2}", tag="ktps")
                    for h in range(2):
                        for ko in range(8):
                            d0 = (dt2 * 2 + h) * P
                            nc.tensor.matmul(ps[:, h, :],
                                             wk_sb[:, ko, d0:d0 + P],
                                             xs_sb[:, ko, :],
                                             start=(ko == 0), stop=(ko == 7))
                    st = stage.tile([P, 2, SH], bf16, name=f"kt_st{dt2}", tag="ktst")
                    nc.vector.tensor_copy(st[:], ps[:])
                    dst = bass.AP(agin, dt2 * (2 * P * SH),
                                  [[SH, P], [P * SH, 2], [1, SH]])
                    nc.sync.dma_start(dst, st[:])

                # v_c: [512 s][1024 d] into agin[KT_ELEMS : ]
                for st_i in range(4):
                    ps = ppsv.tile([P, D], f32, name=f"v_ps{st_i}", tag="vps")
                    for ko in range(8):
                        for dh in range(2):
                            nc.tensor.matmul(
                                ps[:, dh * 512:(dh + 1) * 512],
                                xs_sb[:, ko, st_i * P:(st_i + 1) * P],
                                wv_sb[:, ko, dh * 512:(dh + 1) * 512],
                                start=(ko == 0), stop=(ko == 7))
                    st = stage.tile([P, D], bf16, name=f"v_st{st_i}", tag="vst")
                    nc.vector.tensor_copy(st[:], ps[:])
                    dst = bass.AP(agin, KT_ELEMS + st_i * (P * D),
                                  [[D, P], [1, D]])
                    nc.sync.dma_start(dst, st[:])

                # ---------------- Phase 2: AllGather K/V ----------------
                if 'ag' in parts:
                    nc.gpsimd.collective_compute(
                        "AllGather", mybir.AluOpType.bypass,
                        replica_groups=[list(range(N_CORES))],
                        ins=[agin.ap().opt()],
                        outs=[agout.ap().opt()],
                    )
                else:
                    for sp8 in range(8):
                        off8 = sp8 * (RANK_ELEMS // 8)
                        nc.sync.dma_start(
                            bass.AP(agout, off8, [[1, 1], [2048, RANK_ELEMS // 16384], [1, 2048]]),
                            bass.AP(agin, off8, [[1, 1], [2048, RANK_ELEMS // 16384], [1, 2048]]))

                # qT_c: keep in SBUF [128 dp, 8 dko, 512 i] (overlaps AG)
                qt_sb = qtpool.tile([P, 8, SH], bf16, name="qt_sb")
                for dt2 in range(4):
                    ps = pps.tile([P, 2, SH], f32, name=f"q_ps{dt2}", tag="ktps")
                    for h in range(2):
                        for ko in range(8):
                            d0 = (dt2 * 2 + h) * P
                            nc.tensor.matmul(ps[:, h, :],
                                             wq_sb[:, ko, d0:d0 + P],
                                             xs_sb[:, ko, :],
                                             start=(ko == 0), stop=(ko == 7))
                    nc.vector.tensor_copy(qt_sb[:, 2 * dt2:2 * dt2 + 2, :], ps[:])

            # ---------------- Phase 3: attention ----------------
            ones_sb = consts.tile([P, 1], bf16, name="ones_sb")
            nc.sync.dma_start(ones_sb[:], onesd[:])
            acc_out = accs.tile([P, 4, D], f32, name="acc_out")
            nc.vector.memset(acc_out[:], 0.0)

            with tc.tile_pool(name="ktw", bufs=wbufs) as ktw, \
                 tc.tile_pool(name="vw", bufs=wbufs) as vw, \
                 tc.tile_pool(name="mk", bufs=mkbufs) as mkp, \
                 tc.tile_pool(name="pt", bufs=ptbufs) as ptp, \
                 tc.tile_pool(name="stps", bufs=1, space="PSUM") as stps, \
                 tc.tile_pool(name="pvps", bufs=1, space="PSUM") as pvps, \
                 tc.tile_pool(name="rsps", bufs=1, space="PSUM") as rsps:

                rs_ps = rsps.tile([1, SH], f32, name="rs_ps")

                n_win_total = N_WIN * win_mult
                for wi in range(n_win_total):
                    W = wi % N_WIN
                    first = wi == 0
                    last = wi == n_win_total - 1
                    kt_w = ktw.tile([P, 8, SH], bf16, name=f"kt_w{wi}", tag="ktw")
                    v_w = vw.tile([P, 4, D], bf16, name=f"v_w{wi}", tag="vw")
                    ns = dma_split
                    for sp in range(ns):
                        ko0, kon = sp * (8 // ns), 8 // ns
                        nc.sync.dma_start(
                            kt_w[:, ko0:ko0 + kon, :],
                            bass.AP(agout, W * RANK_ELEMS + ko0 * P * SH,
                                    [[SH, P], [P * SH, kon], [1, SH]]))
                        jo0, jon = sp * (4 // ns), 4 // ns
                        nc.sync.dma_start(
                            v_w[:, jo0:jo0 + jon, :],
                            bass.AP(agout,
                                    W * RANK_ELEMS + KT_ELEMS + jo0 * P * D,
                                    [[D, P], [P * D, jon], [1, D]]))

                    if not pv_split:
                        pv_ps = pvps.tile([P, 2, D], f32, name=f"pv{wi}", tag="pv")
                    pts = []
                    for pair in range(2):
                        if st_split:
                            st_ps = stps.tile([P, 2, SH], f32,
                                              name=f"st{wi}_{pair}", tag="st",
                                              bufs=2)
                        else:
                            st_ps = stps.tile([P, 2, SH], f32,
                                              name=f"st{wi}_{pair}", tag="st")
                        if 'sc' in parts:
                            for js in range(2):
                                jj = pair * 2 + js
                                for ko in range(8):
                                    nc.tensor.matmul(
                                        st_ps[:, js, :],
                                        kt_w[:, ko, jj * P:(jj + 1) * P],
                                        qt_sb[:, ko, :],
                                        start=(ko == 0), stop=(ko == 7))
                        else:
                            nc.vector.memset(st_ps[:], 0.1)
                        pt = ptp.tile([P, 2, SH], bf16, name=f"pt{wi}_{pair}",
                                      tag="pt")
                        if 'act' in parts:
                            nc.scalar.activation(pt[:], st_ps[:],
                                                 mybir.ActivationFunctionType.Exp,
                                                 scale=SCALE)
                        else:
                            nc.vector.tensor_copy(pt[:], st_ps[:])
                        if 'msk' in parts:
                            mk = mkp.tile([P, 2, SH], bf16, name=f"mk{wi}_{pair}",
                                          tag="mk")
                            nc.sync.dma_start(mk[:], maskd[W, pair])
                            nc.vector.tensor_mul(pt[:], pt[:], mk[:])
                        pts.append(pt)
                        if 'rs' in parts:
                            for js in range(2):
                                nc.tensor.matmul(
                                    rs_ps[:], ones_sb[:], pt[:, js, :],
                                    start=(first and pair == 0 and js == 0),
                                    stop=(last and pair == 1 and js == 1))

                    # PV: isub-pair (or per-iblk if pv_split) passes
                    if pv_split and 'pv' in parts:
                        for iblk in range(4):
                            pvq = pvps.tile([P, 1, D], f32,
                                            name=f"pvq{wi}_{iblk}", tag="pv")
                            for pair in range(2):
                                for js in range(2):
                                    for dh in range(2):
                                        nc.tensor.matmul(
                                            pvq[:, 0, dh * 512:(dh + 1) * 512],
                                            pts[pair][:, js,
                                                      iblk * P:(iblk + 1) * P],
                                            v_w[:, pair * 2 + js,
                                                dh * 512:(dh + 1) * 512],
                                            start=(pair == 0 and js == 0),
                                            stop=(pair == 1 and js == 1))
                            nc.vector.tensor_add(acc_out[:, iblk, :],
                                                 acc_out[:, iblk, :],
                                                 pvq[:, 0, :])
                    else:
                      for half in range(2 if 'pv' in parts else 0):
                        for pair in range(2):
                            for js in range(2):
                                for ib in range(2):
                                    iblk = half * 2 + ib
                                    for dh in range(2):
                                        nc.tensor.matmul(
                                            pv_ps[:, ib, dh * 512:(dh + 1) * 512],
                                            pts[pair][:, js,
                                                      iblk * P:(iblk + 1) * P],
                                            v_w[:, pair * 2 + js,
                                                dh * 512:(dh + 1) * 512],
                                            start=(pair == 0 and js == 0),
                                            stop=(pair == 1 and js == 1))
                        for ib in range(2):
                            iblk = half * 2 + ib
                            nc.vector.tensor_add(acc_out[:, iblk, :],
                                                 acc_out[:, iblk, :],
                                                 pv_ps[:, ib, :])

                # ---------------- finalize: divide by row sums ----------------
                if 'rs' not in parts:
                    nc.vector.memset(rs_ps[:], 1.0)
                rs_sb = consts.tile([1, SH], f32, name="rs_sb")
                nc.vector.reciprocal(rs_sb[:], rs_ps[:])
                nc.sync.dma_start(rs_dram.ap(), rs_sb[:])
                recipT = consts.tile([P, 4], f32, name="recipT")
                nc.sync.dma_start(
                    recipT[:],
                    rs_dram.ap().rearrange("o (ib p) -> (o p) ib", p=P))
                o_f32 = accs.tile([P, 4, D], f32, name="o_f32")
                nc.vector.tensor_tensor(
                    o_f32[:], acc_out[:],
                    recipT[:, :, None].to_broadcast((P, 4, D)),
                    mybir.AluOpType.mult)
                nc.sync.dma_start(
                    outd.ap().rearrange("(ib p) d -> p ib d", p=P), o_f32[:])

    nc.compile()
    return nc


def _host_inputs(x, W_query, W_key, W_value):
    bf = ml_dtypes.bfloat16

    def wprep(W):
        return np.ascontiguousarray(
            W.reshape(8, P, D).transpose(1, 0, 2)).astype(bf)

    wq_n, wk_n, wv_n = wprep(W_query), wprep(W_key), wprep(W_value)

    in_maps = []
    for c in range(N_CORES):
        rows = np.arange(SH * c, SH * (c + 1))
        xt = x[rows].T.reshape(8, P, SH).transpose(1, 0, 2)
        xs_n = np.ascontiguousarray(xt).astype(bf)

        # mask[W, pair, p, js, i]: valid iff key (512W + (2*pair+js)*128 + p)
        #                               <= query (512c + i)
        mask = np.zeros((N_WIN, 2, P, 2, SH), dtype=np.float32)
        for Wn in range(N_WIN):
            if Wn < c:
                mask[Wn] = 1.0
            elif Wn == c:
                for pair in range(2):
                    for js in range(2):
                        jj = pair * 2 + js
                        j_rel = jj * P + np.arange(P)[:, None]
                        i_rel = np.arange(SH)[None, :]
                        mask[Wn, pair, :, js, :] = (j_rel <= i_rel)
        in_maps.append({
            "wq": wq_n, "wk": wk_n, "wv": wv_n, "xs": xs_n,
            "mask": mask.astype(bf),
            "ones": np.ones((P, 1), dtype=bf),
        })
    return in_maps


def kernel(x, W_query, W_key, W_value):
    from concourse.bass_utils import run_bass_kernel_spmd

    x = np.asarray(x, dtype=np.float32)
    W_query = np.asarray(W_query, dtype=np.float32)
    W_key = np.asarray(W_key, dtype=np.float32)
    W_value = np.asarray(W_value, dtype=np.float32)

    if "nc" not in _CACHE:
        _CACHE["nc"] = _build()
    nc = _CACHE["nc"]

    in_maps = _host_inputs(x, W_query, W_key, W_value)
    res = run_bass_kernel_spmd(nc, in_maps, core_ids=list(range(N_CORES)))

    out = np.empty((S, D), dtype=np.float32)
    for c in range(N_CORES):
        out[SH * c:SH * (c + 1)] = res.results[c]["out"]
    return out



# revision 9
# speedup vs baseline: 1.5068x; 1.5068x over previous
"""Causal self-attention (single head, S=4096, D=1024) on 8 TRN2 NeuronCores.

Strategy (striped sequence-parallel, causality-exploiting):
  - Core c owns the strided query rows {i : i mod 8 == c} (local index
    l = 0..511, global i = 8l + c) and computes K/V projections for the
    contiguous rows [512c, 512(c+1)); K^T/V are AllGathered in bf16.
  - Because ownership is striped, for key window W (global keys
    [512W, 512W+512)) only local queries l >= 64*W can attend -- the SAME
    static range on every core.  Causality therefore cuts the score/PV
    matmul work to ~56% with a single SPMD program; the per-core +-c row
    wiggle lives entirely in a 64-column mask band per window.
  - Scores are computed TRANSPOSED (S^T[j, l], keys on partitions): softmax
    needs no max-subtraction (scores are O(5)), exp(S^T) feeds the PV matmul
    directly as lhsT, and row sums fall out of a pt @ ones matmul with the
    sums landing on PARTITIONS (no transpose round-trip for the divide).
  - The PE stream is software-pipelined (scores of window W+1 are emitted
    before PV of window W) so the tensor engine never waits on exp/mask.
  - Output rows complete per 128-row l-chunk as soon as its last window is
    accumulated, so the divide + store overlap the remaining windows.
"""

import numpy as np
import ml_dtypes

S = 4096
D = 1024
N_CORES = 8
P = 128
L = 512               # local query rows per core (striped)
N_WIN = 8
KT_ELEMS = D * L      # per-rank kT block elements in AG buffer (kT row-major [D, 512])
V_ELEMS = L * D       # per-rank v block elements ([512, D] row-major)
SCALE = 1.0 / 32.0    # 1/sqrt(D)

_CACHE = {}


def _build(parts=frozenset({'sc', 'pv', 'act', 'msk', 'rs', 'dma', 'acc', 'ag'})):
    import concourse.bass as bass
    import concourse.mybir as mybir
    import concourse.tile as tile
    from concourse import bacc

    bf16 = mybir.dt.bfloat16
    f32 = mybir.dt.float32

    nc = bacc.Bacc("TRN2", target_bir_lowering=False, debug=False,
                   num_devices=N_CORES)

    # ---- per-core I/O ----
    wq = nc.dram_tensor("wq", [P, 8, D], bf16, kind="ExternalInput")
    wk = nc.dram_tensor("wk", [P, 8, D], bf16, kind="ExternalInput")
    wv = nc.dram_tensor("wv", [P, 8, D], bf16, kind="ExternalInput")
    xkv = nc.dram_tensor("xkv", [P, 8, L], bf16, kind="ExternalInput")   # x^T contiguous shard
    xq = nc.dram_tensor("xq", [P, 8, L], bf16, kind="ExternalInput")     # x^T strided shard
    maskd = nc.dram_tensor("mask", [N_WIN, P, 4, 64], bf16, kind="ExternalInput")
    outd = nc.dram_tensor("out", [L, D], f32, kind="ExternalOutput")

    agin_k = nc.dram_tensor("agin_k", [1, KT_ELEMS], bf16)
    agout_k = nc.dram_tensor("agout_k", [1, N_CORES * KT_ELEMS], bf16,
                             addr_space="Shared")
    agin_v = nc.dram_tensor("agin_v", [1, V_ELEMS], bf16)
    agout_v = nc.dram_tensor("agout_v", [1, N_CORES * V_ELEMS], bf16,
                             addr_space="Shared")

    def ag(agin, agout):
        if 'ag' in parts:
            nc.gpsimd.collective_compute(
                "AllGather", mybir.AluOpType.bypass,
                replica_groups=[list(range(N_CORES))],
                ins=[agin.ap().opt()],
                outs=[agout.ap().opt()],
            )
        else:
            # Local stand-in with the same per-core traffic shape.
            n = agin.shape[1]
            for sp in range(2):
                off = sp * (n // 2)
                nc.sync.dma_start(
                    bass.AP(agout, off, [[1, 1], [1, n // 2]]),
                    bass.AP(agin, off, [[1, 1], [1, n // 2]]))

    with tile.TileContext(nc) as tc:
        with tc.tile_pool(name="wpool", bufs=3) as wpool, \
             tc.tile_pool(name="xpool", bufs=2) as xpool, \
             tc.tile_pool(name="qt", bufs=1) as qtpool, \
             tc.tile_pool(name="stage", bufs=3) as stage, \
             tc.tile_pool(name="consts", bufs=1) as consts, \
             tc.tile_pool(name="accs", bufs=1) as accs:

            # ---------------- Phase 1: projections ----------------
            xkv_sb = xpool.tile([P, 8, L], bf16, name="xkv_sb", tag="x")
            xq_sb = xpool.tile([P, 8, L], bf16, name="xq_sb", tag="x")
            wk_sb = wpool.tile([P, 8, D], bf16, name="wk_sb", tag="w")
            wv_sb = wpool.tile([P, 8, D], bf16, name="wv_sb", tag="w")
            wq_sb = wpool.tile([P, 8, D], bf16, name="wq_sb", tag="w")
            for ko in range(8):
                nc.sync.dma_start(xkv_sb[:, ko, :], xkv[:, ko, :])
                nc.sync.dma_start(wk_sb[:, ko, :], wk[:, ko, :])
            for ko in range(8):
                nc.sync.dma_start(wv_sb[:, ko, :], wv[:, ko, :])
                nc.sync.dma_start(xq_sb[:, ko, :], xq[:, ko, :])
            for ko in range(8):
                nc.sync.dma_start(wq_sb[:, ko, :], wq[:, ko, :])

            with tc.tile_pool(name="pps", bufs=2, space="PSUM") as pps, \
                 tc.tile_pool(name="ppsv", bufs=2, space="PSUM") as ppsv:
                # kT_c: [1024 d][512 j] row-major into agin_k
                for dt2 in range(4):          # two 128-d tiles per psum tile
                    ps = pps.tile([P, 2, L], f32, name=f"kt_ps{dt2}", tag="ktps")
                    for h in range(2):
                        for ko in range(8):
                            d0 = (dt2 * 2 + h) * P
                            nc.tensor.matmul(ps[:, h, :],
                                             wk_sb[:, ko, d0:d0 + P],
                                             xkv_sb[:, ko, :],
                                             start=(ko == 0), stop=(ko == 7))
                    st = stage.tile([P, 2, L], bf16, name=f"kt_st{dt2}", tag="ktst")
                    nc.vector.tensor_copy(st[:], ps[:])
                    dst = bass.AP(agin_k, dt2 * (2 * P * L),
                                  [[L, P], [P * L, 2], [1, L]])
                    nc.sync.dma_start(dst, st[:])

                # -------- AllGather K (early, overlaps V/Q projections) ----
                ag(agin_k, agout_k)

                # v_c: [512 j][1024 d] row-major into agin_v
                for st_i in range(4):
                    ps = ppsv.tile([P, D], f32, name=f"v_ps{st_i}", tag="vps")
                    for ko in range(8):
                        for dh in range(2):
                            nc.tensor.matmul(
                                ps[:, dh * 512:(dh + 1) * 512],
                                xkv_sb[:, ko, st_i * P:(st_i + 1) * P],
                                wv_sb[:, ko, dh * 512:(dh + 1) * 512],
                                start=(ko == 0), stop=(ko == 7))
                    st = stage.tile([P, D], bf16, name=f"v_st{st_i}", tag="vst")
                    nc.vector.tensor_copy(st[:], ps[:])
                    dst = bass.AP(agin_v, st_i * (P * D), [[D, P], [1, D]])
                    nc.sync.dma_start(dst, st[:])

                ag(agin_v, agout_v)

                # qT_c: keep in SBUF [128 dp, 8 dko, 512 l] (overlaps AGs)
                qt_sb = qtpool.tile([P, 8, L], bf16, name="qt_sb")
                for dt2 in range(4):
                    ps = pps.tile([P, 2, L], f32, name=f"q_ps{dt2}", tag="ktps")
                    for h in range(2):
                        for ko in range(8):
                            d0 = (dt2 * 2 + h) * P
                            nc.tensor.matmul(ps[:, h, :],
                                             wq_sb[:, ko, d0:d0 + P],
                                             xq_sb[:, ko, :],
                                             start=(ko == 0), stop=(ko == 7))
                    nc.vector.tensor_copy(qt_sb[:, 2 * dt2:2 * dt2 + 2, :], ps[:])

            # ---------------- Phase 2: attention ----------------
            # acc column 1024 holds the running row sums (ones-column of V)
            acc = accs.tile([P, 4, D + 1], f32, name="acc")      # l-chunk accum
            recip_sb = consts.tile([P, 4], f32, name="recip_sb")

            with tc.tile_pool(name="ktw", bufs=3) as ktw, \
                 tc.tile_pool(name="vw", bufs=3) as vw, \
                 tc.tile_pool(name="mk", bufs=2) as mkp, \
                 tc.tile_pool(name="pt", bufs=2) as ptp, \
                 tc.tile_pool(name="outp", bufs=2) as outp, \
                 tc.tile_pool(name="stps", bufs=2, space="PSUM") as stps, \
                 tc.tile_pool(name="pvps", bufs=2, space="PSUM") as pvps:

                kt_tiles = {}
                v_tiles = {}
                pt_tiles = {}

                def dma_win(W):
                    kt_w = ktw.tile([P, 8, L], bf16, name=f"kt_w{W}", tag="ktw")
                    v_w = vw.tile([P, 4, D + 1], bf16, name=f"v_w{W}", tag="vw")
                    nc.vector.memset(v_w[:, :, D:D + 1], 1.0)
                    for sp in range(2):
                        ko0 = sp * 4
                        nc.sync.dma_start(
                            kt_w[:, ko0:ko0 + 4, :],
                            bass.AP(agout_k, W * KT_ELEMS + ko0 * P * L,
                                    [[L, P], [P * L, 4], [1, L]]))
                        js0 = sp * 2
                        nc.sync.dma_start(
                            v_w[:, js0:js0 + 2, 0:D],
                            bass.AP(agout_v, W * V_ELEMS + js0 * P * D,
                                    [[D, P], [P * D, 2], [1, D]]))
                    kt_tiles[W], v_tiles[W] = kt_w, v_w

                def scores(W):
                    l0 = 64 * W
                    lc0 = 128 * (W // 2)
                    lcnt = L - l0
                    pt = ptp.tile([P, 4, L], bf16, name=f"pt{W}", tag="pt")
                    if l0 != lc0:
                        # odd window: zero the half-chunk strip PV overshoots
                        nc.vector.memset(pt[:, :, lc0:l0], 0.0)
                    kt_w = kt_tiles[W]
                    for js in range(4):
                        st_ps = stps.tile([P, lcnt], f32, name=f"st{W}_{js}",
                                          tag="st")
                        if 'sc' in parts:
                            for ko in range(8):
                                nc.tensor.matmul(
                                    st_ps[:],
                                    kt_w[:, ko, js * P:(js + 1) * P],
                                    qt_sb[:, ko, l0:L],
                                    start=(ko == 0), stop=(ko == 7))
                        else:
                            nc.vector.memset(st_ps[:], 0.1)
                        if 'act' in parts:
                            nc.scalar.activation(pt[:, js, l0:L], st_ps[:],
                                                 mybir.ActivationFunctionType.Exp,
                                                 scale=SCALE)
                        else:
                            nc.vector.tensor_copy(pt[:, js, l0:L], st_ps[:])
                    if 'msk' in parts:
                        mk = mkp.tile([P, 4, 64], bf16, name=f"mk{W}", tag="mk")
                        nc.sync.dma_start(mk[:], maskd[W])
                        nc.vector.tensor_mul(pt[:, :, l0:l0 + 64],
                                             pt[:, :, l0:l0 + 64], mk[:])
                    pt_tiles[W] = pt

                def finalize(ci):
                    if 'rs' in parts and 'pv' in parts:
                        nc.vector.reciprocal(recip_sb[:, ci:ci + 1],
                                             acc[:, ci, D:D + 1])
                    else:
                        nc.vector.memset(recip_sb[:, ci:ci + 1], 1.0)
                    o_t = outp.tile([P, D], f32, name=f"o{ci}", tag="o")
                    nc.scalar.activation(o_t[:], acc[:, ci, 0:D],
                                         mybir.ActivationFunctionType.Copy,
                                         scale=recip_sb[:, ci:ci + 1])
                    nc.sync.dma_start(
                        bass.AP(outd, ci * P * D, [[D, P], [1, D]]),
                        o_t[:])

                def pv_rs(W):
                    ci0 = W // 2
                    pt = pt_tiles[W]
                    v_w = v_tiles[W]
                    if 'pv' in parts:
                        for ci in range(ci0, 4):
                            pv = pvps.tile([P, D + 1], f32, name=f"pv{W}_{ci}",
                                           tag="pv")
                            for dh in range(2):
                                for js in range(4):
                                    nc.tensor.matmul(
                                        pv[:, dh * 512:(dh + 1) * 512],
                                        pt[:, js, ci * P:(ci + 1) * P],
                                        v_w[:, js, dh * 512:(dh + 1) * 512],
                                        start=(js == 0), stop=(js == 3))
                            if 'rs' in parts:
                                # row sums ride along in the V ones-column;
                                # its accumulation group owns its own bank
                                for js in range(4):
                                    nc.tensor.matmul(
                                        pv[:, D:D + 1],
                                        pt[:, js, ci * P:(ci + 1) * P],
                                        v_w[:, js, D:D + 1],
                                        start=(js == 0), stop=(js == 3))
                            else:
                                nc.vector.memset(pv[:, D:D + 1], 1.0)
                            if W == 0:
                                nc.vector.tensor_copy(acc[:, ci, :], pv[:])
                            else:
                                nc.vector.tensor_add(acc[:, ci, :],
                                                     acc[:, ci, :], pv[:])
                    # finalize chunks whose last window just accumulated
                    if W % 2 == 1:
                        finalize(W // 2)

                # software pipeline: PE does scores(W+1) before pv(W)
                dma_win(0)
                dma_win(1)
                scores(0)
                for W in range(N_WIN):
                    if W + 2 < N_WIN:
                        dma_win(W + 2)
                    if W + 1 < N_WIN:
                        scores(W + 1)
                    pv_rs(W)

    nc.compile()
    return nc


def _host_inputs(x, W_query, W_key, W_value):
    bf = ml_dtypes.bfloat16

    def wprep(W):
        return np.ascontiguousarray(
            W.reshape(8, P, D).transpose(1, 0, 2)).astype(bf)

    def xprep(rows):
        xt = x[rows].T.reshape(8, P, L).transpose(1, 0, 2)
        return np.ascontiguousarray(xt).astype(bf)

    wq_n, wk_n, wv_n = wprep(W_query), wprep(W_key), wprep(W_value)

    in_maps = []
    for c in range(N_CORES):
        # mask[W, jp, js, lb]: key (512W + js*128 + jp) vs query (8*(64W+lb)+c)
        #   valid iff js*128 + jp <= 8*lb + c
        jj = (np.arange(4)[None, :, None] * P +
              np.arange(P)[:, None, None])          # [jp, js, 1]
        lb = np.arange(64)[None, None, :]           # [1, 1, lb]
        mask = (jj <= 8 * lb + c).astype(np.float32)   # [P, 4, 64]
        mask = np.broadcast_to(mask, (N_WIN, P, 4, 64))
        in_maps.append({
            "wq": wq_n, "wk": wk_n, "wv": wv_n,
            "xkv": xprep(np.arange(L * c, L * (c + 1))),
            "xq": xprep(np.arange(L) * 8 + c),
            "mask": np.ascontiguousarray(mask).astype(bf),
        })
    return in_maps


def kernel(x, W_query, W_key, W_value):
    from concourse.bass_utils import run_bass_kernel_spmd

    x = np.asarray(x, dtype=np.float32)
    W_query = np.asarray(W_query, dtype=np.float32)
    W_key = np.asarray(W_key, dtype=np.float32)
    W_value = np.asarray(W_value, dtype=np.float32)

    if "nc" not in _CACHE:
        _CACHE["nc"] = _build()
    nc = _CACHE["nc"]

    in_maps = _host_inputs(x, W_query, W_key, W_value)
    res = run_bass_kernel_spmd(nc, in_maps, core_ids=list(range(N_CORES)))

    out = np.empty((S, D), dtype=np.float32)
    for c in range(N_CORES):
        out[np.arange(L) * 8 + c] = res.results[c]["out"]
    return out


# revision 25
# speedup vs baseline: 1.5375x; 1.0204x over previous
"""Causal self-attention (single head, S=4096, D=1024) on 8 TRN2 NeuronCores.

Strategy (striped sequence-parallel, causality-exploiting):
  - Core c owns the strided query rows {i : i mod 8 == c} (local index
    l = 0..511, global i = 8l + c) and computes K/V projections for the
    contiguous rows [512c, 512(c+1)); K^T/V are AllGathered in bf16.
  - Because ownership is striped, for key window W (global keys
    [512W, 512W+512)) only local queries l >= 64*W can attend -- the SAME
    static range on every core.  Causality therefore cuts the score/PV
    matmul work to ~56% with a single SPMD program; the per-core +-c row
    wiggle lives entirely in a 64-column mask band per window.
  - Scores are computed TRANSPOSED (S^T[j, l], keys on partitions): softmax
    needs no max-subtraction (scores are O(5)), exp(S^T) feeds the PV matmul
    directly as lhsT, and row sums fall out of a pt @ ones matmul with the
    sums landing on PARTITIONS (no transpose round-trip for the divide).
  - The PE stream is software-pipelined (scores of window W+1 are emitted
    before PV of window W) so the tensor engine never waits on exp/mask.
  - Output rows complete per 128-row l-chunk as soon as its last window is
    accumulated, so the divide + store overlap the remaining windows.
"""

import numpy as np
import ml_dtypes

S = 4096
D = 1024
N_CORES = 8
P = 128
L = 512               # local query rows per core (striped)
N_WIN = 8
KT_ELEMS = D * L      # per-rank kT block elements in AG buffer (kT row-major [D, 512])
V_ELEMS = L * D       # per-rank v block elements ([512, D] row-major)
SCALE = 1.0 / 32.0    # 1/sqrt(D)

_CACHE = {}


def _build(parts=frozenset({'sc', 'pv', 'act', 'msk', 'rs', 'dma', 'acc', 'ag'})):
    import concourse.bass as bass
    import concourse.mybir as mybir
    import concourse.tile as tile
    from concourse import bacc

    bf16 = mybir.dt.bfloat16
    f32 = mybir.dt.float32

    nc = bacc.Bacc("TRN2", target_bir_lowering=False, debug=False,
                   num_devices=N_CORES)

    # ---- per-core I/O ----
    wq = nc.dram_tensor("wq", [P, 8, D], bf16, kind="ExternalInput")
    wk = nc.dram_tensor("wk", [P, 8, D], bf16, kind="ExternalInput")
    wv = nc.dram_tensor("wv", [P, 8, D], bf16, kind="ExternalInput")
    xkv = nc.dram_tensor("xkv", [P, 8, L], bf16, kind="ExternalInput")   # x^T contiguous shard
    xq = nc.dram_tensor("xq", [P, 8, L], bf16, kind="ExternalInput")     # x^T strided shard
    maskd = nc.dram_tensor("mask", [N_WIN, P, 4, 64], bf16, kind="ExternalInput")
    outd = nc.dram_tensor("out", [L, D], f32, kind="ExternalOutput")

    agin_k = nc.dram_tensor("agin_k", [1, KT_ELEMS], bf16)
    agout_k = nc.dram_tensor("agout_k", [1, N_CORES * KT_ELEMS], bf16,
                             addr_space="Shared")
    agin_v = nc.dram_tensor("agin_v", [1, V_ELEMS], bf16)
    agout_v = nc.dram_tensor("agout_v", [1, N_CORES * V_ELEMS], bf16,
                             addr_space="Shared")

    def ag(agin, agout):
        if 'ag' in parts:
            nc.gpsimd.collective_compute(
                "AllGather", mybir.AluOpType.bypass,
                replica_groups=[list(range(N_CORES))],
                ins=[agin.ap().opt()],
                outs=[agout.ap().opt()],
            )
        else:
            # Local stand-in with the same per-core traffic shape.
            n = agin.shape[1]
            for sp in range(2):
                off = sp * (n // 2)
                nc.sync.dma_start(
                    bass.AP(agout, off, [[1, 1], [1, n // 2]]),
                    bass.AP(agin, off, [[1, 1], [1, n // 2]]))

    with tile.TileContext(nc) as tc:
        with tc.tile_pool(name="wpool", bufs=3) as wpool, \
             tc.tile_pool(name="xpool", bufs=2) as xpool, \
             tc.tile_pool(name="qt", bufs=4) as qtpool, \
             tc.tile_pool(name="stage", bufs=3) as stage, \
             tc.tile_pool(name="consts", bufs=1) as consts, \
             tc.tile_pool(name="accs", bufs=1) as accs:

            # ---------------- Phase 1: projections ----------------
            xkv_sb = xpool.tile([P, 8, L], bf16, name="xkv_sb", tag="x")
            xq_sb = xpool.tile([P, 8, L], bf16, name="xq_sb", tag="x")
            wk_sb = wpool.tile([P, 8, D], bf16, name="wk_sb", tag="w")
            wv_sb = wpool.tile([P, 8, D], bf16, name="wv_sb", tag="w")
            wq_sb = wpool.tile([P, 8, D], bf16, name="wq_sb", tag="w")
            for ko in range(8):
                nc.sync.dma_start(xkv_sb[:, ko, :], xkv[:, ko, :])
                nc.sync.dma_start(wk_sb[:, ko, :], wk[:, ko, :])
            for ko in range(8):
                nc.sync.dma_start(wv_sb[:, ko, :], wv[:, ko, :])
                nc.sync.dma_start(xq_sb[:, ko, :], xq[:, ko, :])
            for ko in range(8):
                nc.sync.dma_start(wq_sb[:, ko, :], wq[:, ko, :])

            with tc.tile_pool(name="pps", bufs=2, space="PSUM") as pps, \
                 tc.tile_pool(name="ppsv", bufs=2, space="PSUM") as ppsv:
                # kT_c: [1024 d][512 j] row-major into agin_k
                for dt2 in range(4):          # two 128-d tiles per psum tile
                    ps = pps.tile([P, 2, L], f32, name=f"kt_ps{dt2}", tag="ktps")
                    for h in range(2):
                        for ko in range(8):
                            d0 = (dt2 * 2 + h) * P
                            nc.tensor.matmul(ps[:, h, :],
                                             wk_sb[:, ko, d0:d0 + P],
                                             xkv_sb[:, ko, :],
                                             start=(ko == 0), stop=(ko == 7))
                    st = stage.tile([P, 2, L], bf16, name=f"kt_st{dt2}", tag="ktst")
                    nc.vector.tensor_copy(st[:], ps[:])
                    dst = bass.AP(agin_k, dt2 * (2 * P * L),
                                  [[L, P], [P * L, 2], [1, L]])
                    nc.sync.dma_start(dst, st[:])

                # -------- AllGather K (early, overlaps V/Q projections) ----
                ag(agin_k, agout_k)

                # v_c: [512 j][1024 d] row-major into agin_v
                for st_i in range(4):
                    ps = ppsv.tile([P, D], f32, name=f"v_ps{st_i}", tag="vps")
                    for ko in range(8):
                        for dh in range(2):
                            nc.tensor.matmul(
                                ps[:, dh * 512:(dh + 1) * 512],
                                xkv_sb[:, ko, st_i * P:(st_i + 1) * P],
                                wv_sb[:, ko, dh * 512:(dh + 1) * 512],
                                start=(ko == 0), stop=(ko == 7))
                    st = stage.tile([P, D], bf16, name=f"v_st{st_i}", tag="vst")
                    nc.vector.tensor_copy(st[:], ps[:])
                    dst = bass.AP(agin_v, st_i * (P * D), [[D, P], [1, D]])
                    nc.sync.dma_start(dst, st[:])

                ag(agin_v, agout_v)

                # qT_c: four SBUF tiles [128 dp, 2 dko, 512 l] so later
                # readers only wait on the copy they actually need
                qt_sbs = [qtpool.tile([P, 2, L], bf16, name=f"qt_sb{d}",
                                      tag=f"qt{d}") for d in range(4)]
                for dt2 in range(4):
                    ps = pps.tile([P, 2, L], f32, name=f"q_ps{dt2}", tag="ktps")
                    for h in range(2):
                        for ko in range(8):
                            d0 = (dt2 * 2 + h) * P
                            nc.tensor.matmul(ps[:, h, :],
                                             wq_sb[:, ko, d0:d0 + P],
                                             xq_sb[:, ko, :],
                                             start=(ko == 0), stop=(ko == 7))
                    nc.vector.tensor_copy(qt_sbs[dt2][:], ps[:])

            # ---------------- Phase 2: attention ----------------
            # acc column 1024 holds the running row sums (ones-column of V)
            acc = accs.tile([P, 4, D + 1], f32, name="acc")      # l-chunk accum
            recip_sb = consts.tile([P, 4], f32, name="recip_sb")

            with tc.tile_pool(name="ktw", bufs=3) as ktw, \
                 tc.tile_pool(name="vw", bufs=3) as vw, \
                 tc.tile_pool(name="mk", bufs=2) as mkp, \
                 tc.tile_pool(name="pt", bufs=2) as ptp, \
                 tc.tile_pool(name="outp", bufs=2) as outp, \
                 tc.tile_pool(name="pvps", bufs=2, space="PSUM") as pvps, \
                 tc.tile_pool(name="stps", bufs=2, space="PSUM") as stps:

                kt_tiles = {}
                v_tiles = {}
                pt_tiles = {}
                pv_pair = {}
                o_tiles = {}

                def dma_win_kt(W):
                    kt_w = ktw.tile([P, 8, L], bf16, name=f"kt_w{W}", tag="ktw")
                    for sp in range(2):
                        ko0 = sp * 4
                        nc.sync.dma_start(
                            kt_w[:, ko0:ko0 + 4, :],
                            bass.AP(agout_k, W * KT_ELEMS + ko0 * P * L,
                                    [[L, P], [P * L, 4], [1, L]]))
                    kt_tiles[W] = kt_w

                def dma_win_v(W):
                    v_w = vw.tile([P, 4, D + 1], bf16, name=f"v_w{W}", tag="vw")
                    nc.vector.memset(v_w[:, :, D:D + 1], 1.0)
                    for sp in range(2):
                        js0 = sp * 2
                        nc.sync.dma_start(
                            v_w[:, js0:js0 + 2, 0:D],
                            bass.AP(agout_v, W * V_ELEMS + js0 * P * D,
                                    [[D, P], [P * D, 2], [1, D]]))
                    v_tiles[W] = v_w

                def dma_win(W):
                    dma_win_kt(W)
                    dma_win_v(W)

                def scores(W):
                    l0 = 64 * W
                    lc0 = 128 * (W // 2)
                    lcnt = L - l0
                    pt = ptp.tile([P, 4, L], bf16, name=f"pt{W}", tag="pt")
                    if l0 != lc0:
                        # odd window: zero the half-chunk strip PV overshoots
                        nc.vector.memset(pt[:, :, lc0:l0], 0.0)
                    kt_w = kt_tiles[W]
                    for js in range(4):
                        st_ps = stps.tile([P, lcnt], f32, name=f"st{W}_{js}",
                                          tag="st")
                        if 'sc' in parts:
                            for ko in range(8):
                                nc.tensor.matmul(
                                    st_ps[:],
                                    kt_w[:, ko, js * P:(js + 1) * P],
                                    qt_sbs[ko // 2][:, ko % 2, l0:L],
                                    start=(ko == 0), stop=(ko == 7))
                        else:
                            nc.vector.memset(st_ps[:], 0.1)
                        if 'act' in parts:
                            nc.scalar.activation(pt[:, js, l0:L], st_ps[:],
                                                 mybir.ActivationFunctionType.Exp,
                                                 scale=SCALE)
                        else:
                            nc.vector.tensor_copy(pt[:, js, l0:L], st_ps[:])
                    if 'msk' in parts:
                        mk = mkp.tile([P, 4, 64], bf16, name=f"mk{W}", tag="mk")
                        nc.sync.dma_start(mk[:], maskd[W])
                        nc.vector.tensor_mul(pt[:, :, l0:l0 + 64],
                                             pt[:, :, l0:l0 + 64], mk[:])
                    pt_tiles[W] = pt

                def finalize_half(ci, half):
                    # half 1 = cols [512, 1024) plus the rs column (emitted
                    # first so the reciprocal starts early); half 0 = [0, 512)
                    if half == 1:
                        if 'rs' in parts and 'pv' in parts:
                            nc.vector.reciprocal(recip_sb[:, ci:ci + 1],
                                                 acc[:, ci, D:D + 1])
                        else:
                            nc.vector.memset(recip_sb[:, ci:ci + 1], 1.0)
                        o_t = outp.tile([P, D], f32, name=f"o{ci}", tag="o")
                        o_tiles[ci] = o_t
                    else:
                        o_t = o_tiles[ci]
                    d0 = half * 512
                    nc.scalar.activation(o_t[:, d0:d0 + 512],
                                         acc[:, ci, d0:d0 + 512],
                                         mybir.ActivationFunctionType.Copy,
                                         scale=recip_sb[:, ci:ci + 1])
                    nc.sync.dma_start(
                        bass.AP(outd, ci * P * D + d0, [[D, P], [1, 512]]),
                        o_t[:, d0:d0 + 512])

                def pv_rs(W):
                    # windows >= 4 touch at most two l-chunks, so window
                    # pairs (4,5) and (6,7) accumulate directly in PSUM
                    ci0 = W // 2
                    pt = pt_tiles[W]
                    v_w = v_tiles[W]
                    paired = W >= 4
                    first = (not paired) or (W % 2 == 0)
                    last = (not paired) or (W % 2 == 1)
                    if 'pv' not in parts:
                        return
                    for ci in range(ci0, 4):
                        if first:
                            pv = pvps.tile([P, D + 1], f32, name=f"pv{W}_{ci}",
                                           tag="pv")
                            pv_pair[ci] = pv
                        else:
                            pv = pv_pair[ci]
                        if 'rs' in parts:
                            # row sums ride along in the V ones-column;
                            # its accumulation group owns its own bank
                            # (emitted first so its stop lands early)
                            for js in range(4):
                                nc.tensor.matmul(
                                    pv[:, D:D + 1],
                                    pt[:, js, ci * P:(ci + 1) * P],
                                    v_w[:, js, D:D + 1],
                                    start=(first and js == 0),
                                    stop=(last and js == 3))
                        elif first:
                            nc.vector.memset(pv[:, D:D + 1], 1.0)
                        for dh in range(2):
                            for js in range(4):
                                nc.tensor.matmul(
                                    pv[:, dh * 512:(dh + 1) * 512],
                                    pt[:, js, ci * P:(ci + 1) * P],
                                    v_w[:, js, dh * 512:(dh + 1) * 512],
                                    start=(first and js == 0),
                                    stop=(last and js == 3))
                        if not last:
                            continue
                        if W == 0:
                            nc.vector.tensor_copy(acc[:, ci, :], pv[:])
                        elif W == 2 * ci + 1 and ci == 3 and 'rs' in parts:
                            # final chunk: rs bank stops before the dh banks,
                            # so the reciprocal and first store launch while
                            # the last PV matmuls still run
                            nc.vector.tensor_add(acc[:, ci, D:D + 1],
                                                 acc[:, ci, D:D + 1],
                                                 pv[:, D:D + 1])
                            nc.vector.reciprocal(recip_sb[:, ci:ci + 1],
                                                 acc[:, ci, D:D + 1])
                            o_t = outp.tile([P, D], f32, name=f"o{ci}", tag="o")
                            for half in range(2):
                                d0 = half * 512
                                nc.vector.tensor_add(acc[:, ci, d0:d0 + 512],
                                                     acc[:, ci, d0:d0 + 512],
                                                     pv[:, d0:d0 + 512])
                                nc.scalar.activation(
                                    o_t[:, d0:d0 + 512],
                                    acc[:, ci, d0:d0 + 512],
                                    mybir.ActivationFunctionType.Copy,
                                    scale=recip_sb[:, ci:ci + 1])
                                nc.sync.dma_start(
                                    bass.AP(outd, ci * P * D + d0,
                                            [[D, P], [1, 512]]),
                                    o_t[:, d0:d0 + 512])
                        elif W == 2 * ci + 1:
                            # chunk complete: split the last add so the
                            # divide/store pipeline starts on the rs half
                            nc.vector.tensor_add(acc[:, ci, 512:D + 1],
                                                 acc[:, ci, 512:D + 1],
                                                 pv[:, 512:D + 1])
                            finalize_half(ci, 1)
                            nc.vector.tensor_add(acc[:, ci, 0:512],
                                                 acc[:, ci, 0:512],
                                                 pv[:, 0:512])
                            finalize_half(ci, 0)
                        else:
                            nc.vector.tensor_add(acc[:, ci, :],
                                                 acc[:, ci, :], pv[:])

                # software pipeline: PE does scores(W+1) before pv(W)
                dma_win(0)
                dma_win(1)
                scores(0)
                for W in range(N_WIN):
                    if W + 2 < N_WIN:
                        dma_win(W + 2)
                    if W + 1 < N_WIN:
                        scores(W + 1)
                    pv_rs(W)

    nc.compile()
    return nc


def _host_inputs(x, W_query, W_key, W_value):
    bf = ml_dtypes.bfloat16

    def wprep(W):
        return np.ascontiguousarray(
            W.reshape(8, P, D).transpose(1, 0, 2)).astype(bf)

    def xprep(rows):
        xt = x[rows].T.reshape(8, P, L).transpose(1, 0, 2)
        return np.ascontiguousarray(xt).astype(bf)

    wq_n, wk_n, wv_n = wprep(W_query), wprep(W_key), wprep(W_value)

    in_maps = []
    for c in range(N_CORES):
        # mask[W, jp, js, lb]: key (512W + js*128 + jp) vs query (8*(64W+lb)+c)
        #   valid iff js*128 + jp <= 8*lb + c
        jj = (np.arange(4)[None, :, None] * P +
              np.arange(P)[:, None, None])          # [jp, js, 1]
        lb = np.arange(64)[None, None, :]           # [1, 1, lb]
        mask = (jj <= 8 * lb + c).astype(np.float32)   # [P, 4, 64]
        mask = np.broadcast_to(mask, (N_WIN, P, 4, 64))
        in_maps.append({
            "wq": wq_n, "wk": wk_n, "wv": wv_n,
            "xkv": xprep(np.arange(L * c, L * (c + 1))),
            "xq": xprep(np.arange(L) * 8 + c),
            "mask": np.ascontiguousarray(mask).astype(bf),
        })
    return in_maps


def kernel(x, W_query, W_key, W_value):
    from concourse.bass_utils import run_bass_kernel_spmd

    x = np.asarray(x, dtype=np.float32)
    W_query = np.asarray(W_query, dtype=np.float32)
    W_key = np.asarray(W_key, dtype=np.float32)
    W_value = np.asarray(W_value, dtype=np.float32)

    if "nc" not in _CACHE:
        _CACHE["nc"] = _build()
    nc = _CACHE["nc"]

    in_maps = _host_inputs(x, W_query, W_key, W_value)
    res = run_bass_kernel_spmd(nc, in_maps, core_ids=list(range(N_CORES)))

    out = np.empty((S, D), dtype=np.float32)
    for c in range(N_CORES):
        out[np.arange(L) * 8 + c] = res.results[c]["out"]
    return out
